# revision 2
# baseline (speedup 1.0000x reference)
"""Trainium2 Bass kernel for nn_FCGAT (fully-connected GAT block).

Math: the reference computes
    h      = x @ W + bW
    scores = LeakyReLU(s_i[:,None] + s_j[None,:] + a_b)
    a      = softmax(scores, axis=-1)
    out    = relu(einsum('nkj,nkd->nkd', a, h))
The einsum contracts `a` over j only, i.e. multiplies h elementwise by the
softmax row-sums, which are exactly 1.  So out == relu(x @ W + bW) up to
float rounding.  The kernel therefore runs a memory-bound fused
GEMM+bias+relu, data-parallel over the batch dim N across 8 NeuronCores.

Precision: the problem tolerance (rel_err < 2e-2) leaves room to move the
HBM-bound I/O in fp16 instead of fp32 — x and W are cast to fp16 on the
host, the matmul accumulates in fp32 PSUM, bias stays fp32, and the relu
output is stored as fp16 and upcast on the host while unsharding.  This
halves both directions of HBM traffic (the kernel is purely DMA-bound) for
a measured end-to-end error of ~3.6e-4.

Device layout (per core, rows = 8*1024 = 8192):
  The host hands each core its x shard transposed (xT: [128 feat, 8192
  rows] fp16) so the contraction dim lands on SBUF partitions with no
  on-device transposes.  W stays stationary in the PE array; each matmul
  streams 512 rows into one PSUM bank (fp32).  Bias+relu+PSUM->SBUF(fp16)
  runs as ONE instruction per matmul, alternating between the scalar
  engine (activation) and the vector engine (tensor_scalar add+max) so
  neither engine's serial time sits on the store critical path.

DMA plan (measured on the target cores):  reads sustain ~325 GB/s, writes
~345 GB/s, but concurrent bidirectional traffic tops out at ~305 GB/s
combined regardless of queue assignment or phasing — so the floor for
2 MiB in + 2 MiB out is ~13.7 us/core and the schedule only needs to keep
the port busy: 2 x 1 MiB load chunks on the SP HWDGE ring, 2 x 1 MiB
stores on the ACT HWDGE ring (HWDGE avoids the SWDGE Q7 descriptor-
emission path, which measurably interferes with the split-engine
activations).  PE warm-up matmuls release the HAM clock throttle before
the first real matmul.
"""

import os

import numpy as np

import concourse.bacc as bacc
import concourse.mybir as mybir
import concourse.tile as tile
from concourse.bass_utils import run_bass_kernel_spmd

N, K, D1, D2 = 64, 1024, 128, 128
NCORES = 8
ROWS = (N // NCORES) * K  # 8192 rows per core
CH = 4096  # rows per DMA chunk (1 MiB in fp16)
MM = 512  # moving rows per matmul (= one fp32 PSUM bank)

F32 = mybir.dt.float32
F16 = mybir.dt.float16

_nc_cache = None

# Results of the most recent hardware run (BassKernelResults); lets a test
# harness read exec_time_ns when KERNEL_TRACE=1 is set.
LAST_RESULTS = None


def _build_nc(repeat=1):
    """Build the per-core Bass kernel.

    ``repeat`` re-runs the identical pipeline that many times inside one
    NEFF (same DRAM in/out) — used only for slope-based HW timing.
    """
    nc = bacc.Bacc("TRN2", target_bir_lowering=False, debug=False)

    xt = nc.dram_tensor("xT", [D1, ROWS], F16, kind="ExternalInput").ap()
    w_d = nc.dram_tensor("Wh", [D1, D2], F16, kind="ExternalInput").ap()
    b_d = nc.dram_tensor("bias", [D2, 1], F32, kind="ExternalInput").ap()
    outt = nc.dram_tensor("outT", [D2, ROWS], F16, kind="ExternalOutput").ap()

    with tile.TileContext(nc) as tc:
        with (
            tc.tile_pool(name="const", bufs=1) as cpool,
            tc.tile_pool(name="xin", bufs=2) as xpool,
            tc.tile_pool(name="oout", bufs=2) as opool,
            tc.tile_pool(name="ps", bufs=6, space="PSUM") as pspool,
            tc.tile_pool(name="warm", bufs=1, space="PSUM") as wpool,
        ):
            # Constants go FIRST on the SP HWDGE ring: they are tiny but
            # gate the first matmul/activation, so they must land before
            # the bulk x loads monopolize the HBM port.
            w_s = cpool.tile([D1, D2], F16)
            nc.sync.dma_start(w_s[:], w_d)
            b_s = cpool.tile([D2, 1], F32)
            nc.sync.dma_start(b_s[:], b_d)

            # PE warm-up: chained dummy matmuls on zeros release the HAM
            # clock throttle before the first real matmul arrives.  The
            # dummy activation forces the Relu table load off the critical
            # path.
            warm = cpool.tile([D1, 256], F16)
            nc.gpsimd.memset(warm[:], 0.0)
            nc.scalar.activation(
                warm[:], warm[:], mybir.ActivationFunctionType.Relu, bias=0.0
            )
            wps = wpool.tile([D2, 256], F32)
            NWARM = 4
            for i in range(NWARM):
                nc.tensor.matmul(
                    wps[:],
                    lhsT=warm[:, :D2],
                    rhs=warm[:],
                    start=(i == 0),
                    stop=(i == NWARM - 1),
                )

            for _r in range(repeat):
                for ci in range(ROWS // CH):
                    pos = ci * CH
                    xin = xpool.tile([D1, CH], F16, tag="xin")
                    # loads on the SP HWDGE ring
                    nc.sync.dma_start(xin[:], xt[:, pos : pos + CH])
                    oout = opool.tile([D2, CH], F16, tag="oout")
                    for m in range(CH // MM):
                        ps = pspool.tile([D2, MM], F32, tag="ps")
                        nc.tensor.matmul(
                            ps[:],
                            lhsT=w_s[:],
                            rhs=xin[:, m * MM : (m + 1) * MM],
                            start=True,
                            stop=True,
                        )
                        if m % 2 == 1:
                            # vector engine: out = max(psum + bias, 0)
                            nc.vector.tensor_scalar(
                                oout[:, m * MM : (m + 1) * MM],
                                ps[:],
                                b_s[:],
                                0.0,
                                mybir.AluOpType.add,
                                mybir.AluOpType.max,
                            )
                        else:
                            # scalar engine: out = relu(psum + bias)
                            nc.scalar.activation(
                                oout[:, m * MM : (m + 1) * MM],
                                ps[:],
                                mybir.ActivationFunctionType.Relu,
                                bias=b_s[:],
                            )
                    # stores on the ACT HWDGE ring: never queue behind the
                    # loads on the SP ring, and skip the SWDGE Q7 path
                    nc.scalar.dma_start(outt[:, pos : pos + CH], oout[:])

    nc.compile()
    return nc


def _make_in_maps(np_inputs):
    x = np.asarray(np_inputs["x"], dtype=np.float32).reshape(N * K, D1)
    xt16 = np.asarray(x.T, dtype=np.float16)  # [D1, N*K]
    w16 = np.asarray(np_inputs["W"], dtype=np.float16)
    b32 = np.ascontiguousarray(
        np.asarray(np_inputs["bW"], dtype=np.float32).reshape(D2, 1)
    )
    in_maps = []
    for i in range(NCORES):
        in_maps.append(
            {
                "xT": np.ascontiguousarray(xt16[:, i * ROWS : (i + 1) * ROWS]),
                "Wh": w16,
                "bias": b32,
            }
        )
    return in_maps


def kernel(x, W, bW, a_w=None, a_b=None, **_unused):
    global _nc_cache, LAST_RESULTS
    if _nc_cache is None:
        _nc_cache = _build_nc()
    nc = _nc_cache

    in_maps = _make_in_maps({"x": x, "W": W, "bW": bW})

    trace = bool(os.environ.get("KERNEL_TRACE"))
    try:
        res = run_bass_kernel_spmd(nc, in_maps, list(range(NCORES)), trace=trace)
    except ModuleNotFoundError:
        # Chipless axon client without the NTFF profile hook package —
        # rerun without tracing.
        os.environ["BASS_NEVER_TRACE"] = "1"
        res = run_bass_kernel_spmd(nc, in_maps, list(range(NCORES)), trace=False)
    LAST_RESULTS = res

    out = np.concatenate(
        [np.asarray(res.results[i]["outT"]).T for i in range(NCORES)], axis=0
    )
    return np.ascontiguousarray(out.reshape(N, K, D2).astype(np.float32))


# revision 4
# speedup vs baseline: 1.0737x; 1.0737x over previous
"""Trainium2 Bass kernel for nn_FCGAT (fully-connected GAT block).

Math: the reference computes
    h      = x @ W + bW
    scores = LeakyReLU(s_i[:,None] + s_j[None,:] + a_b)
    a      = softmax(scores, axis=-1)
    out    = relu(einsum('nkj,nkd->nkd', a, h))
The einsum contracts `a` over j only, i.e. multiplies h elementwise by the
softmax row-sums, which are exactly 1.  So out == relu(x @ W + bW) up to
float rounding.  The kernel therefore runs a memory-bound fused
GEMM+bias+relu, data-parallel over the batch dim N across 8 NeuronCores.

Precision: the problem tolerance (rel_err < 2e-2) leaves room to move the
HBM-bound I/O in fp16 instead of fp32 — x and W are cast to fp16 on the
host, the matmul accumulates in fp32 PSUM, bias stays fp32, and the relu
output is stored as fp16 and upcast on the host while unsharding.  This
halves both directions of HBM traffic (the kernel is purely DMA-bound) for
a measured end-to-end error of ~3.6e-4.

Device layout (per core, rows = 8*1024 = 8192):
  The host hands each core its x shard transposed (xT: [128 feat, 8192
  rows] fp16) so the contraction dim lands on SBUF partitions with no
  on-device transposes.  W stays stationary in the PE array; each matmul
  streams 512 rows into one PSUM bank (fp32).  Bias+relu+PSUM->SBUF(fp16)
  runs as ONE instruction per matmul, alternating between the scalar
  engine (activation) and the vector engine (tensor_scalar add+max) so
  neither engine's serial time sits on the store critical path.

DMA plan (measured on the target cores):  reads sustain ~325 GB/s, writes
~345 GB/s, but concurrent bidirectional traffic tops out at ~305 GB/s
combined regardless of queue assignment or phasing — so the floor for
2 MiB in + 2 MiB out is ~13.7 us/core and the schedule only needs to keep
the port busy: 2 x 1 MiB load chunks on the SP HWDGE ring, 2 x 1 MiB
stores on the ACT HWDGE ring (HWDGE avoids the SWDGE Q7 descriptor-
emission path, which measurably interferes with the split-engine
activations).  PE warm-up matmuls release the HAM clock throttle before
the first real matmul.
"""

import os

import numpy as np

import concourse.bacc as bacc
import concourse.mybir as mybir
import concourse.tile as tile
from concourse.bass_utils import run_bass_kernel_spmd

N, K, D1, D2 = 64, 1024, 128, 128
NCORES = 8
ROWS = (N // NCORES) * K  # 8192 rows per core
CH = 4096  # rows per DMA chunk (1 MiB in fp16)
MM = 512  # moving rows per matmul (= one fp32 PSUM bank)

F32 = mybir.dt.float32
F16 = mybir.dt.float16

_nc_cache = None

# Results of the most recent hardware run (BassKernelResults); lets a test
# harness read exec_time_ns when KERNEL_TRACE=1 is set.
LAST_RESULTS = None


def _build_nc(repeat=1):
    """Build the per-core Bass kernel.

    ``repeat`` re-runs the identical pipeline that many times inside one
    NEFF (same DRAM in/out) — used only for slope-based HW timing.
    """
    nc = bacc.Bacc("TRN2", target_bir_lowering=False, debug=False)

    xt = nc.dram_tensor("xT", [D1, ROWS], F16, kind="ExternalInput").ap()
    w_d = nc.dram_tensor("Wh", [D1, D2], F16, kind="ExternalInput").ap()
    b_d = nc.dram_tensor("bias", [D2, 1], F32, kind="ExternalInput").ap()
    outt = nc.dram_tensor("outT", [D2, ROWS], F16, kind="ExternalOutput").ap()

    with tile.TileContext(nc) as tc:
        with (
            tc.tile_pool(name="const", bufs=1) as cpool,
            tc.tile_pool(name="xin", bufs=3) as xpool,
            tc.tile_pool(name="oout", bufs=3) as opool,
            tc.tile_pool(name="ps", bufs=6, space="PSUM") as pspool,
            tc.tile_pool(name="warm", bufs=1, space="PSUM") as wpool,
        ):
            # Constants go FIRST on the SP HWDGE ring: they are tiny but
            # gate the first matmul/activation, so they must land before
            # the bulk x loads monopolize the HBM port.
            w_s = cpool.tile([D1, D2], F16)
            nc.sync.dma_start(w_s[:], w_d)
            b_s = cpool.tile([D2, 1], F32)
            nc.sync.dma_start(b_s[:], b_d)

            # PE warm-up: chained dummy matmuls on zeros release the HAM
            # clock throttle before the first real matmul arrives.  The
            # dummy activation forces the Relu table load off the critical
            # path.
            warm = cpool.tile([D1, 256], F16)
            nc.gpsimd.memset(warm[:], 0.0)
            nc.scalar.activation(
                warm[:], warm[:], mybir.ActivationFunctionType.Relu, bias=0.0
            )
            wps = wpool.tile([D2, 256], F32)
            NWARM = 4
            for i in range(NWARM):
                nc.tensor.matmul(
                    wps[:],
                    lhsT=warm[:, :D2],
                    rhs=warm[:],
                    start=(i == 0),
                    stop=(i == NWARM - 1),
                )

            for _r in range(repeat):
                for ci in range(ROWS // CH):
                    pos = ci * CH
                    xin = xpool.tile([D1, CH], F16, tag="xin")
                    # loads on the SP HWDGE ring
                    nc.sync.dma_start(xin[:], xt[:, pos : pos + CH])
                    oout = opool.tile([D2, CH], F16, tag="oout")
                    for m in range(CH // MM):
                        ps = pspool.tile([D2, MM], F32, tag="ps")
                        nc.tensor.matmul(
                            ps[:],
                            lhsT=w_s[:],
                            rhs=xin[:, m * MM : (m + 1) * MM],
                            start=True,
                            stop=True,
                        )
                        # 5/3 DVE-heavy split: DVE is ~1.6x faster per
                        # [128,512] pass than ACT, so this balances the two
                        # elementwise pipelines (and measures ~200ns faster
                        # than a 4/4 split with bufs=3)
                        if "DADDADDA"[m % 8] == "D":
                            # vector engine: out = max(psum + bias, 0)
                            nc.vector.tensor_scalar(
                                oout[:, m * MM : (m + 1) * MM],
                                ps[:],
                                b_s[:],
                                0.0,
                                mybir.AluOpType.add,
                                mybir.AluOpType.max,
                            )
                        else:
                            # scalar engine: out = relu(psum + bias)
                            nc.scalar.activation(
                                oout[:, m * MM : (m + 1) * MM],
                                ps[:],
                                mybir.ActivationFunctionType.Relu,
                                bias=b_s[:],
                            )
                    # stores on the ACT HWDGE ring: never queue behind the
                    # loads on the SP ring, and skip the SWDGE Q7 path
                    nc.scalar.dma_start(outt[:, pos : pos + CH], oout[:])

    nc.compile()
    return nc


def _make_in_maps(np_inputs):
    x = np.asarray(np_inputs["x"], dtype=np.float32).reshape(N * K, D1)
    xt16 = np.asarray(x.T, dtype=np.float16)  # [D1, N*K]
    w16 = np.asarray(np_inputs["W"], dtype=np.float16)
    b32 = np.ascontiguousarray(
        np.asarray(np_inputs["bW"], dtype=np.float32).reshape(D2, 1)
    )
    in_maps = []
    for i in range(NCORES):
        in_maps.append(
            {
                "xT": np.ascontiguousarray(xt16[:, i * ROWS : (i + 1) * ROWS]),
                "Wh": w16,
                "bias": b32,
            }
        )
    return in_maps


def kernel(x, W, bW, a_w=None, a_b=None, **_unused):
    global _nc_cache, LAST_RESULTS
    if _nc_cache is None:
        _nc_cache = _build_nc()
    nc = _nc_cache

    in_maps = _make_in_maps({"x": x, "W": W, "bW": bW})

    trace = bool(os.environ.get("KERNEL_TRACE"))
    try:
        res = run_bass_kernel_spmd(nc, in_maps, list(range(NCORES)), trace=trace)
    except ModuleNotFoundError:
        # Chipless axon client without the NTFF profile hook package —
        # rerun without tracing.
        os.environ["BASS_NEVER_TRACE"] = "1"
        res = run_bass_kernel_spmd(nc, in_maps, list(range(NCORES)), trace=False)
    LAST_RESULTS = res

    out = np.concatenate(
        [np.asarray(res.results[i]["outT"]).T for i in range(NCORES)], axis=0
    )
    return np.ascontiguousarray(out.reshape(N, K, D2).astype(np.float32))
